# revision 1
# baseline (speedup 1.0000x reference)
"""Trainium2 Bass kernel for the spiral-conv mesh autoencoder (nn_AE_45810121179173).

Data-parallel over batch: core i runs the full network for batch element i.
- all gathers (spiral convs + pools) via SWDGE indirect DMA from DRAM tables,
  chunked into large multi-row gathers to amortize descriptor-gen overhead
- pools converted host-side to race-free degree-sorted padded-CSR gather +
  weighted tree-sum (downstream spiral indices remapped by the sort)
- convs: gather rows (vertex-major) -> PE transpose to feature-major ->
  PSUM-accumulated bf16 matmuls -> bias + ELU -> next table
Self-contained: hardcodes shapes; needs numpy + concourse runtime only.
"""
import sys

for _p in ("/opt/trn_rl_repo", "/root/.axon_site/_ro/trn_rl_repo"):
    if _p not in sys.path:
        sys.path.insert(0, _p)

import numpy as np

V = [40000, 10000, 2500, 625, 160]
L = 9
LAT = 256
B = 8
P = 128
TCONV = 512
G = 4
CHUNK_BYTES = 2 << 20  # gather staging budget per chunk


def _pow2_up(x):
    n = 1
    while n < x:
        n *= 2
    return n


def _prep_pool(row, col, val, v_out):
    nnz = len(row)
    deg = np.bincount(row, minlength=v_out)
    perm = np.argsort(-deg, kind="stable").astype(np.int64)
    inv = np.empty(v_out, np.int64)
    inv[perm] = np.arange(v_out)
    order = np.argsort(row, kind="stable")
    col_s, val_s = col[order].astype(np.int64), np.asarray(val)[order].astype(np.float32)
    starts = np.zeros(v_out + 1, np.int64)
    np.cumsum(deg, out=starts[1:])
    ntiles = (v_out + P - 1) // P
    widths, idx_l, val_l = [], [], []
    for t in range(ntiles):
        dests = perm[t * P:(t + 1) * P]
        k = deg[dests]
        W = _pow2_up(max(1, int(k.max()) if len(k) else 1))
        npd = len(dests)
        mask = np.arange(W)[None, :] < k[:, None]
        flat = np.minimum(starts[dests][:, None] + np.arange(W)[None, :], max(nnz - 1, 0))
        I = np.zeros((P, W), np.int64)
        A = np.zeros((P, W), np.float32)
        I[:npd] = np.where(mask, col_s[flat], 0)
        A[:npd] = np.where(mask, val_s[flat], 0.0)
        widths.append(W)
        idx_l.append(I)
        val_l.append(A)
    return dict(perm=perm, inv=inv, widths=widths,
                idx=np.concatenate(idx_l, 1).astype(np.int32),
                val=np.concatenate(val_l, 1).astype(np.float32), ntiles=ntiles)


def _prep_conv(sp, inv):
    """[128, ntiles*36]; [p, t*36 + gr*9 + j] = sp'[t*512 + gr*128 + p, j]."""
    Vq = sp.shape[0]
    sp_r = (inv[sp] if inv is not None else np.asarray(sp)).astype(np.int64)
    ntiles = (Vq + TCONV - 1) // TCONV
    pad = ntiles * TCONV - Vq
    if pad:
        sp_r = np.concatenate([sp_r, np.zeros((pad, L), np.int64)], 0)
    arr = sp_r.reshape(ntiles, G, P, L)
    idx = arr.transpose(2, 0, 1, 3).reshape(P, ntiles * L * G)
    return idx.astype(np.int32), ntiles


def preprocess(d):
    pr = {}
    for l in range(4):
        pr[f"pd{l}"] = _prep_pool(np.asarray(d[f"dr{l}"]), np.asarray(d[f"dc{l}"]),
                                  np.asarray(d[f"dv{l}"]), V[l + 1])
        pr[f"pu{l}"] = _prep_pool(np.asarray(d[f"ur{l}"]), np.asarray(d[f"uc{l}"]),
                                  np.asarray(d[f"uv{l}"]), V[l])
    sps = [np.asarray(d[f"sp{l}"]) for l in range(4)]
    pr["ce0"] = _prep_conv(sps[0], None)
    for l in range(1, 4):
        pr[f"ce{l}"] = _prep_conv(sps[l], pr[f"pd{l-1}"]["inv"])
    for l in range(4):
        pr[f"cd{l}"] = _prep_conv(sps[l], pr[f"pu{l}"]["inv"])
    pr["cf"] = _prep_conv(sps[0], None)
    Wenc = np.asarray(d["Wenc"]).reshape(V[4], 64, LAT)
    pr["Wenc_p"] = np.ascontiguousarray(
        Wenc[pr["pd3"]["perm"]].reshape(V[4] * 64, LAT))
    return pr


def build(pr, weights, dbg=False):
    from concourse import bacc, bass, tile, mybir
    import ml_dtypes

    f32, bf16, i32 = mybir.dt.float32, mybir.dt.bfloat16, mybir.dt.int32
    nc = bacc.Bacc(None, target_bir_lowering=False)
    ext = {}

    def ein(name, arr, dtype):
        h = nc.dram_tensor(name, list(arr.shape), dtype, kind="ExternalInput")
        if dtype == bf16:
            ext[name] = np.asarray(arr).astype(ml_dtypes.bfloat16)
        elif dtype == i32:
            ext[name] = np.asarray(arr).astype(np.int32)
        else:
            ext[name] = np.asarray(arr).astype(np.float32)
        return h

    x_h = nc.dram_tensor("x", [V[0], 3], f32, kind="ExternalInput")
    conv_idx = {}
    for cname in ["ce0", "ce1", "ce2", "ce3", "cd3", "cd2", "cd1", "cd0", "cf"]:
        idx, nt = pr[cname]
        conv_idx[cname] = (ein(f"idx_{cname}", idx, i32), nt)
    pool_meta = {}
    for pname in ["pd0", "pd1", "pd2", "pd3", "pu3", "pu2", "pu1", "pu0"]:
        pp = pr[pname]
        pool_meta[pname] = dict(
            idx=ein(f"pidx_{pname}", pp["idx"], i32),
            val=ein(f"pval_{pname}", pp["val"], f32),
            widths=pp["widths"], ntiles=pp["ntiles"])
    weights = dict(weights)
    weights["ident"] = np.eye(P, dtype=np.float32)
    weights["identb"] = np.eye(P, dtype=np.float32)
    wdict = {}
    for nm, arr in weights.items():
        wdict[nm] = ein(nm, arr, bf16 if (nm.startswith("W") or nm == "identb") else f32)

    def tpad(v, m):
        return ((v + m - 1) // m) * m

    tabs = {}
    for nm, (r, c) in dict(
            te0=(tpad(V[0], TCONV), 32), tp0=(tpad(V[1], P), 32),
            te1=(tpad(V[1], TCONV), 32), tp1=(tpad(V[2], P), 32),
            te2=(tpad(V[2], TCONV), 32), tp2=(tpad(V[3], P), 32),
            te3=(tpad(V[3], TCONV), 64), tp3=(tpad(V[4], P), 64),
            thd=(V[4], 64),
            tu3=(tpad(V[3], P), 64), td3=(tpad(V[3], TCONV), 64),
            tu2=(tpad(V[2], P), 64), td2=(tpad(V[2], TCONV), 32),
            tu1=(tpad(V[1], P), 32), td1=(tpad(V[1], TCONV), 32),
            tu0=(tpad(V[0], P), 32), td0=(tpad(V[0], TCONV), 32)).items():
        tabs[nm] = nc.dram_tensor(f"tab_{nm}", [r, c], bf16,
                                  kind="ExternalOutput" if dbg else "Internal")
    out_h = nc.dram_tensor("out", [V[0], 3], f32, kind="ExternalOutput")

    AT = mybir.ActivationFunctionType
    OP = mybir.AluOpType

    with tile.TileContext(nc) as tc:
        from contextlib import ExitStack
        es = ExitStack()
        wp = es.enter_context(tc.tile_pool(name="wp", bufs=1))
        gst = es.enter_context(tc.tile_pool(name="gst", bufs=2))
        ixp = es.enter_context(tc.tile_pool(name="ixp", bufs=2))
        rhp = es.enter_context(tc.tile_pool(name="rhp", bufs=8))
        elp = es.enter_context(tc.tile_pool(name="elp", bufs=2))
        pps = es.enter_context(tc.tile_pool(name="pps", bufs=2))
        stp = es.enter_context(tc.tile_pool(name="stp", bufs=2))
        pst = es.enter_context(tc.tile_pool(name="pst", bufs=2, space="PSUM"))
        pso = es.enter_context(tc.tile_pool(name="pso", bufs=2, space="PSUM"))
        psm = es.enter_context(tc.tile_pool(name="psm", bufs=1, space="PSUM"))

        idf = wp.tile([P, P], f32, tag="idf")
        nc.sync.dma_start(out=idf[:], in_=wdict["ident"][:])
        idb = wp.tile([P, P], bf16, tag="idb")
        nc.sync.dma_start(out=idb[:], in_=wdict["identb"][:])

        wenc_sb = wp.tile([P, 80 * LAT], bf16, tag="wenc")
        nc.sync.dma_start(out=wenc_sb[:].rearrange("p (k n) -> p k n", n=LAT),
                          in_=wdict["Wenc_p"][:].rearrange("(k p) n -> p k n", p=P))
        benc_sb = wp.tile([1, LAT], f32, tag="benc")
        nc.sync.dma_start(out=benc_sb[:], in_=wdict["benc"][None, :])
        bdec_sb = wp.tile([1, 10240], f32, tag="bdec")
        nc.sync.dma_start(out=bdec_sb[:], in_=wdict["bdec"][None, :])

        wsb, bsb = {}, {}
        for nm in ["We0", "We1", "We2", "We3", "Wd0", "Wd1", "Wd2", "Wd3", "Wf"]:
            h = wdict[nm]
            rows, cout = h.shape
            nchunk = (rows + P - 1) // P
            t = wp.tile([min(P, rows), nchunk * cout], bf16, tag=f"w_{nm}")
            for ck in range(nchunk):
                r0, r1 = ck * P, min(rows, (ck + 1) * P)
                nc.sync.dma_start(out=t[: r1 - r0, ck * cout:(ck + 1) * cout],
                                  in_=h[r0:r1, :])
            wsb[nm] = (t, rows, cout)
        for nm in ["be0", "be1", "be2", "be3", "bd0", "bd1", "bd2", "bd3", "bf"]:
            h = wdict[nm]
            t = wp.tile([h.shape[0], 1], f32, tag=f"b_{nm}")
            nc.sync.dma_start(out=t[:], in_=h[:, None])
            bsb[nm] = t

        def conv(cname, src_tab, cin, src_dtype, wname, bname, dst_tab, Vq,
                 elu=True, final=False):
            idx_h, ntiles = conv_idx[cname]
            wt, wrows, cout = wsb[wname]
            bias = bsb[bname]
            spj = max(1, P // cin)
            nchunks = (L + spj - 1) // spj
            chunks = [(b * spj, min(L, (b + 1) * spj)) for b in range(nchunks)]
            esz = 2 if src_dtype == bf16 else 4
            NT = max(1, min(16, CHUNK_BYTES // (36 * cin * esz * P)))
            ident = idb if src_dtype == bf16 else idf
            o_dt = f32 if final else bf16
            for c0 in range(0, ntiles, NT):
                nt = min(NT, ntiles - c0)
                ix = ixp.tile([P, 16 * 36], i32, tag="cidx")
                nc.sync.dma_start(out=ix[:, : nt * 36],
                                  in_=idx_h[:, c0 * 36:(c0 + nt) * 36])
                g = gst.tile([P, NT * 36 * cin], src_dtype, tag="cg")
                for sI in range(nt * 36):
                    nc.gpsimd.indirect_dma_start(
                        out=g[:, sI * cin:(sI + 1) * cin], out_offset=None,
                        in_=src_tab[:],
                        in_offset=bass.IndirectOffsetOnAxis(
                            ap=ix[:, sI:sI + 1], axis=0))

                ost = stp.tile([P, NT * G * cout], o_dt, tag="cost")
                for t in range(nt):
                    rhs_t = []
                    for bI, (j0, j1) in enumerate(chunks):
                        cb = (j1 - j0) * cin
                        tp = pst.tile([P, 512], src_dtype, tag="tp")
                        for gr in range(G):
                            base = ((t * G + gr) * L + j0) * cin
                            nc.tensor.transpose(
                                out=tp[:cb, gr * P:(gr + 1) * P],
                                in_=g[:, base: base + cb],
                                identity=ident[:, :])
                        r = rhp.tile([P, 512], bf16, tag="rhs")
                        nc.vector.tensor_copy(out=r[:cb, :], in_=tp[:cb, :])
                        rhs_t.append((r, cb))
                    po = pso.tile([64, 512], f32, tag="po")
                    for bI in range(nchunks):
                        r, cb = rhs_t[bI]
                        nc.tensor.matmul(
                            out=po[:cout, :], lhsT=wt[:cb, bI * cout:(bI + 1) * cout],
                            rhs=r[:cb, :], start=(bI == 0), stop=(bI == nchunks - 1))
                    a1 = elp.tile([64, 512], f32, tag="a1")
                    nc.scalar.activation(out=a1[:cout, :], in_=po[:cout, :],
                                         func=AT.Identity, bias=bias[:, :], scale=1.0)
                    if elu:
                        m = elp.tile([64, 512], f32, tag="m")
                        nc.vector.tensor_scalar_min(out=m[:cout, :],
                                                    in0=a1[:cout, :], scalar1=0.0)
                        e = elp.tile([64, 512], f32, tag="e")
                        nc.scalar.activation(out=e[:cout, :], in_=m[:cout, :],
                                             func=AT.Exp)
                        src_o = elp.tile([64, 512], bf16, tag="ofin")
                        nc.vector.scalar_tensor_tensor(
                            out=src_o[:cout, :], in0=e[:cout, :], scalar=-1.0,
                            in1=a1[:cout, :], op0=OP.add, op1=OP.max)
                    else:
                        src_o = a1
                    tb = pst.tile([P, 512], o_dt, tag="tp")
                    for gr in range(G):
                        nc.tensor.transpose(
                            out=tb[:, gr * cout:(gr + 1) * cout],
                            in_=src_o[:cout, gr * P:(gr + 1) * P],
                            identity=(idf if final else idb)[:cout, :cout])
                    nc.vector.tensor_copy(
                        out=ost[:, t * G * cout:(t + 1) * G * cout],
                        in_=tb[:, : G * cout])
                if not final:
                    nc.sync.dma_start(
                        out=dst_tab[c0 * TCONV:(c0 + nt) * TCONV, :]
                            .rearrange("(q p) c -> p q c", p=P),
                        in_=ost[:, : nt * G * cout]
                            .rearrange("p (q c) -> p q c", c=cout))
                else:
                    v0 = c0 * TCONV
                    nv = min(Vq - v0, nt * TCONV)
                    full_q = nv // P
                    if full_q:
                        nc.sync.dma_start(
                            out=dst_tab[v0: v0 + full_q * P, :]
                                .rearrange("(q p) c -> p q c", p=P),
                            in_=ost[:, : full_q * cout]
                                .rearrange("p (q c) -> p q c", c=cout))
                    rem = nv - full_q * P
                    if rem:
                        nc.sync.dma_start(
                            out=dst_tab[v0 + full_q * P: v0 + nv, :],
                            in_=ost[:rem, full_q * cout:(full_q + 1) * cout])

        def pool(pname, src_tab, C, dst_tab):
            meta = pool_meta[pname]
            widths, ntiles = meta["widths"], meta["ntiles"]
            idx_h, val_h = meta["idx"], meta["val"]
            offs = np.concatenate([[0], np.cumsum(widths)]).astype(int)
            nt_cap = 2048 // C
            t0 = 0
            while t0 < ntiles:
                t1, wsum = t0, 0
                while (t1 < ntiles and t1 - t0 < nt_cap
                       and (wsum + widths[t1]) * C * P * 2 <= CHUNK_BYTES):
                    wsum += widths[t1]
                    t1 += 1
                nt = t1 - t0
                ix = ixp.tile([P, 256], i32, tag="pidx")
                vl = ixp.tile([P, 256], f32, tag="pval")
                nc.sync.dma_start(out=ix[:, :wsum], in_=idx_h[:, offs[t0]:offs[t1]])
                nc.sync.dma_start(out=vl[:, :wsum], in_=val_h[:, offs[t0]:offs[t1]])
                g = gst.tile([P, min(CHUNK_BYTES // (2 * P), 8192)], bf16, tag="cg")
                for wI in range(wsum):
                    nc.gpsimd.indirect_dma_start(
                        out=g[:, wI * C:(wI + 1) * C], out_offset=None,
                        in_=src_tab[:],
                        in_offset=bass.IndirectOffsetOnAxis(
                            ap=ix[:, wI:wI + 1], axis=0))
                ost = stp.tile([P, 2048], bf16, tag="post")
                woff = 0
                for t in range(t0, t1):
                    W = widths[t]
                    s = pps.tile([P, 1024], f32, tag="ps")
                    nc.vector.tensor_tensor(
                        out=s[:, : W * C].rearrange("p (w c) -> p w c", c=C),
                        in0=g[:, woff * C:(woff + W) * C]
                            .rearrange("p (w c) -> p w c", c=C),
                        in1=vl[:, woff:woff + W][:, :, None]
                            .to_broadcast([P, W, C]),
                        op=OP.mult)
                    h = W
                    while h > 1:
                        h //= 2
                        nc.vector.tensor_tensor(
                            out=s[:, : h * C], in0=s[:, : h * C],
                            in1=s[:, h * C: 2 * h * C], op=OP.add)
                    nc.vector.tensor_copy(
                        out=ost[:, (t - t0) * C:(t - t0 + 1) * C], in_=s[:, :C])
                    woff += W
                nc.sync.dma_start(
                    out=dst_tab[t0 * P: t1 * P, :].rearrange("(q p) c -> p q c", p=P),
                    in_=ost[:, : nt * C].rearrange("p (q c) -> p q c", c=C))
                t0 = t1

        # ---------------- network ----------------
        conv("ce0", x_h, 3, f32, "We0", "be0", tabs["te0"], V[0])
        pool("pd0", tabs["te0"], 32, tabs["tp0"])
        conv("ce1", tabs["tp0"], 32, bf16, "We1", "be1", tabs["te1"], V[1])
        pool("pd1", tabs["te1"], 32, tabs["tp1"])
        conv("ce2", tabs["tp1"], 32, bf16, "We2", "be2", tabs["te2"], V[2])
        pool("pd2", tabs["te2"], 32, tabs["tp2"])
        conv("ce3", tabs["tp2"], 32, bf16, "We3", "be3", tabs["te3"], V[3])
        pool("pd3", tabs["te3"], 64, tabs["tp3"])

        h4a = wp.tile([P, 64], bf16, tag="h4a")
        h4b = wp.tile([P, 64], bf16, tag="h4b")
        nc.sync.dma_start(out=h4a[:], in_=tabs["tp3"][0:P, :])
        nc.sync.dma_start(out=h4b[:32, :], in_=tabs["tp3"][P:160, :])
        h4T = wp.tile([64, 160], bf16, tag="h4T")
        tpa = psm.tile([64, P], bf16, tag="mtp")
        nc.tensor.transpose(out=tpa[:, :], in_=h4a[:, :], identity=idb[:, :])
        nc.vector.tensor_copy(out=h4T[:, 0:P], in_=tpa[:, :])
        tpb = psm.tile([64, P], bf16, tag="mtp")
        nc.tensor.transpose(out=tpb[:, :32], in_=h4b[:32, :], identity=idb[:32, :32])
        nc.vector.tensor_copy(out=h4T[:, P:160], in_=tpb[:, :32])
        fl = wp.tile([P, 80], bf16, tag="fl")
        nc.vector.tensor_copy(out=fl[0:64, :], in_=h4T[:, 0:160:2])
        nc.vector.tensor_copy(out=fl[64:P, :], in_=h4T[:, 1:160:2])
        zps = psm.tile([1, LAT], f32, tag="zps")
        w3 = wenc_sb[:].rearrange("p (k n) -> p k n", n=LAT)
        for k in range(80):
            nc.tensor.matmul(out=zps[:, :], lhsT=fl[:, k:k + 1], rhs=w3[:, k, :],
                             start=(k == 0), stop=(k == 79))
        z_sb = wp.tile([1, LAT], bf16, tag="z_sb")
        nc.vector.tensor_tensor(out=z_sb[:], in0=zps[:, :], in1=benc_sb[:],
                                op=OP.add)
        zT = wp.tile([P, 2], bf16, tag="zT")
        for k2 in range(2):
            tz = psm.tile([P, 2], bf16, tag="mtpz")
            nc.tensor.transpose(out=tz[:, 0:1], in_=z_sb[:, k2 * P:(k2 + 1) * P],
                                identity=idb[:1, :1])
            nc.vector.tensor_copy(out=zT[:, k2:k2 + 1], in_=tz[:, 0:1])
        hdec_sb = wp.tile([1, 10240], bf16, tag="hdec")
        for n in range(20):
            wdc = ixp.tile([P, 2 * 512], bf16, tag="wdc")
            nc.sync.dma_start(
                out=wdc[:].rearrange("p (k c) -> p k c", c=512),
                in_=wdict["Wdec"][:, n * 512:(n + 1) * 512]
                    .rearrange("(k p) c -> p k c", p=P))
            hp = psm.tile([1, 512], f32, tag="hp")
            for k2 in range(2):
                nc.tensor.matmul(out=hp[:, :], lhsT=zT[:, k2:k2 + 1],
                                 rhs=wdc[:, k2 * 512:(k2 + 1) * 512],
                                 start=(k2 == 0), stop=(k2 == 1))
            nc.vector.tensor_tensor(out=hdec_sb[:, n * 512:(n + 1) * 512],
                                    in0=hp[:, :],
                                    in1=bdec_sb[:, n * 512:(n + 1) * 512], op=OP.add)
        nc.sync.dma_start(out=tabs["thd"][:, :].rearrange("v c -> (v c)")[None, :],
                          in_=hdec_sb[:, :])

        pool("pu3", tabs["thd"], 64, tabs["tu3"])
        conv("cd3", tabs["tu3"], 64, bf16, "Wd3", "bd3", tabs["td3"], V[3])
        pool("pu2", tabs["td3"], 64, tabs["tu2"])
        conv("cd2", tabs["tu2"], 64, bf16, "Wd2", "bd2", tabs["td2"], V[2])
        pool("pu1", tabs["td2"], 32, tabs["tu1"])
        conv("cd1", tabs["tu1"], 32, bf16, "Wd1", "bd1", tabs["td1"], V[1])
        pool("pu0", tabs["td1"], 32, tabs["tu0"])
        conv("cd0", tabs["tu0"], 32, bf16, "Wd0", "bd0", tabs["td0"], V[0])
        conv("cf", tabs["td0"], 32, bf16, "Wf", "bf", out_h, V[0],
             elu=False, final=True)
        es.close()

    if not nc.is_finalized():
        nc.finalize()
    return nc, ext


def _weights_dict(d, pr):
    w = {}
    for l in range(4):
        for nm in (f"We{l}", f"be{l}", f"Wd{l}", f"bd{l}"):
            w[nm] = np.asarray(d[nm], np.float32)
    for nm in ("Wf", "bf", "benc", "Wdec", "bdec"):
        w[nm] = np.asarray(d[nm], np.float32)
    w["Wenc_p"] = np.asarray(pr["Wenc_p"], np.float32)
    return w


def _run(inputs, trace=False, **kw):
    d = {k: np.asarray(v) for k, v in inputs.items()}
    pr = preprocess(d)
    nc, ext = build(pr, _weights_dict(d, pr))
    from concourse.bass_utils import run_bass_kernel_spmd
    in_maps = []
    for i in range(B):
        m = dict(ext)
        m["x"] = np.asarray(d["x"][i], np.float32)
        in_maps.append(m)
    res = run_bass_kernel_spmd(nc, in_maps, core_ids=list(range(B)),
                               trace=trace, **kw)
    out = np.stack([np.asarray(r["out"], np.float32) for r in res.results], 0)
    return out, res


def kernel(**inputs):
    return _run(inputs)[0]

